# revision 13
# baseline (speedup 1.0000x reference)
"""CQAttention Trainium2 kernel (8-core data parallel), v2.

Math (per example):
    S[i,j] = C@w_c [i] + Q@w_q [j] + (C*w_mul)@Q^T [i,j] + bias
    S1 = softmax_j(where(Qmask==0, -1e9, S))
    S2 = softmax_i(where(Cmask==0, -1e9, S))
    A  = S1 @ Q
    Bm = S1 @ S2^T @ C
    out = concat([C, A, C*A, C*Bm], axis=-1)

Key identities:
  - softmax shift-invariance: `bias` drops out; per-row offsets drop out of
    S1; per-column offsets drop out of S2.
  - With Qm'[d,j] = w_mul[d]*Q[j,d] + w_c[d] (host-packed, folds s0 into the
    score matmul) and bias1[j] = (Q@w_q)[j] + qneg[j] (host-packed):
        E^T[j,i] = exp(Qm'^T@C^T + bias1[j])    one matmul per example.
  - T' = S2^T@C normalized per row j is INVARIANT to any per-j scaling of
    the weights, so the S2 path can reuse E^T's values: the [Lc-part, Lq]
    layout needed for the Lc-contraction is produced by an XBAR DMA
    transpose of E^T (eu[p,t,j] = E^T[j, 128t+p]), not a second matmul.
    Masked-j columns give c[j]=0; +eps before the reciprocal keeps T'
    finite (=0) there, and those rows are killed by E^T[j,:]=0 in abm.
  - Row-major masked C for the T' contraction is built on-chip: XBAR
    transpose of C^T (fp16) -> crow, ones column memset, then one
    broadcast multiply by cm per example gives [cm*C | cm] in bf16.
  - abm per Lc-tile: [A_raw | Bm_raw | r] = E_tile^T.T @ [Q | T' | 1].
    Raw values + r are written out; the host divides by r and forms the
    C*A / C*Bm products during f32 assembly (elementwise O(Lc*D), same
    class as the host-side packing work).

Precision: scores fp16 (f32 PSUM), exp-weights bf16, outputs bf16 raw.
"""

import os
import sys
from contextlib import ExitStack

import ml_dtypes
import numpy as np

for _p in ("/opt/trn_rl_repo", "/root/.axon_site/_ro/trn_rl_repo"):
    if os.path.isdir(_p) and _p not in sys.path:
        sys.path.append(_p)

import concourse.bass as bass
import concourse.tile as tile
from concourse import bacc, mybir
from concourse.bass import ds, ts
from concourse.bass_utils import run_bass_kernel_spmd

F32 = mybir.dt.float32
FP16 = mybir.dt.float16
BF16 = mybir.dt.bfloat16
AF = mybir.ActivationFunctionType
ALU = mybir.AluOpType

N_CORES = 8
B, LC, LQ, D = 64, 1024, 128, 128
B_LOC = B // N_CORES  # 8 examples per core
NT = LC // 128  # 8 Lc tiles of 128


def _build_graph():
    nc = bacc.Bacc("TRN2", target_bir_lowering=False, debug=False)

    CT = nc.dram_tensor("CT", [B_LOC, D, LC], FP16, kind="ExternalInput").ap()
    QM = nc.dram_tensor("QM", [D, B_LOC * LQ], FP16, kind="ExternalInput").ap()
    QS = nc.dram_tensor("QS", [LQ, B_LOC * D], BF16, kind="ExternalInput").ap()
    # host-packed masked C, p-major: [e, p, t*130+x] = (cm*C)[128t+p, x] | cm | 0
    CMB = nc.dram_tensor("CMB", [B_LOC, 128, NT * 130], BF16, kind="ExternalInput").ap()
    B1 = nc.dram_tensor("B1", [LQ, B_LOC], F32, kind="ExternalInput").ap()
    # per-tile raw rows: OUT[e][m, t*257+n] = [A_raw | Bm_raw | r][128t+m, n]
    OUT = nc.dram_tensor("OUT", [B_LOC, 128, NT * 257], BF16, kind="ExternalOutput").ap()

    with tile.TileContext(nc) as tc:
        with ExitStack() as ctx:
            ep = ctx.enter_context

            const = ep(tc.tile_pool(name="const", bufs=1))
            p_ct = ep(tc.tile_pool(name="ct", bufs=B_LOC))
            p_cxb = ep(tc.tile_pool(name="cxb", bufs=4))  # pair tiles
            p_eq = ep(tc.tile_pool(name="eq", bufs=4))
            p_eu = ep(tc.tile_pool(name="eu", bufs=4))
            p_rhs = ep(tc.tile_pool(name="rhs", bufs=B_LOC))
            p_stg = ep(tc.tile_pool(name="stg", bufs=3))
            p_small = ep(tc.tile_pool(name="small", bufs=24))

            pp_e1 = ep(tc.tile_pool(name="pp_e1", bufs=2, space="PSUM"))
            pp_traw = ep(tc.tile_pool(name="pp_traw", bufs=2, space="PSUM"))
            pp_abm = ep(tc.tile_pool(name="pp_abm", bufs=2, space="PSUM"))

            qm_all = const.tile([D, B_LOC * LQ], FP16)
            nc.sync.dma_start(qm_all, QM)

            # PE warmup during the DMA head: dense matmuls flip HAM and start
            # the p-state ramp before real work arrives.
            warm_w = const.tile([128, 512], BF16)
            nc.vector.memset(warm_w, 1.0)
            for _ in range(4):
                warm_ps = pp_e1.tile([128, 512], F32, tag="pe1")
                nc.tensor.matmul(warm_ps[:, 0:256], lhsT=warm_w[:, 0:128], rhs=warm_w[:, 0:256])

            cts, cxbs, eqs, eus, rhss, stgs = {}, {}, {}, {}, {}, {}

            # ---- loads on the two hwdge queues only (swdge desc-gen on
            # gpsimd is slow and serializes with its compute ops) ----
            for e in range(B_LOC):
                ct = p_ct.tile([128, LC], FP16, tag="ct", name=f"ct_{e}")
                (nc.sync if e < 4 else nc.scalar).dma_start(ct, CT[e])
                cts[e] = ct
            qs_all = const.tile([LQ, B_LOC, D], BF16)
            nc.sync.dma_start(qs_all, QS.rearrange("p (e d) -> p e d", d=D))
            b1_sb = const.tile([LQ, B_LOC], F32)
            nc.sync.dma_start(b1_sb, B1)
            for pr in range(B_LOC // 2):
                cxb = p_cxb.tile([128, 2, NT * 130], BF16, tag="cxb", name=f"cxb_{pr}")
                nc.scalar.dma_start(
                    cxb, CMB.rearrange("(q e) p x -> q p e x", e=2)[pr]
                )
                cxbs[2 * pr] = cxb[:, 0, :]
                cxbs[2 * pr + 1] = cxb[:, 1, :]
            for e in range(B_LOC):
                # abm rhs = [Q | T' | 1]; Q block + ones col filled early
                rhs = p_rhs.tile([128, 260], BF16, tag="rhs", name=f"rhs_{e}")
                nc.gpsimd.tensor_copy(rhs[:, 0:128], qs_all[:, e, :])
                nc.gpsimd.memset(rhs[:, 256:257], 1.0)
                rhss[e] = rhs

            eq_batches = {}

            def emit_e1(e):
                # eq batched 4 examples per tile so one XBAR transpose covers
                # all four (transpose issue rate on sync would otherwise gate
                # the traw phase)
                if e % 2 == 0:
                    eq_batches[e // 2] = p_eq.tile(
                        [128, 2, LC], BF16, tag="eq", name=f"eqb_{e // 2}"
                    )
                eq = eq_batches[e // 2][:, e % 2, :]
                for h in range(2):
                    ps = pp_e1.tile([128, 512], F32, tag="pe1", name=f"e1ps_{e}_{h}")
                    nc.tensor.matmul(
                        ps, lhsT=qm_all[:, ts(e, LQ)], rhs=cts[e][:, ts(h, 512)]
                    )
                    nc.scalar.activation(
                        eq[:, ds(512 * h, 512)],
                        ps,
                        func=AF.Exp,
                        bias=b1_sb[:, e : e + 1],
                        scale=1.0,
                    )
                eqs[e] = eq
                if e % 2 == 1:
                    # eu[p, u, j] = E^T[j, 128u+p] over the 2-example batch
                    eu = p_eu.tile([128, 2 * NT, 128], BF16, tag="eu", name=f"eub_{e // 2}")
                    nc.sync.dma_start_transpose(
                        eu, eq_batches[e // 2].rearrange("p a x -> p (a x)")
                    )
                    for ee in (e - 1, e):
                        eus[ee] = eu[:, NT * (ee % 2) : NT * (ee % 2) + NT, :]

            def emit_traw(e):
                traw_ps = pp_traw.tile([128, 132], F32, tag="ptraw", name=f"traw_{e}")
                for t in range(NT):
                    nc.tensor.matmul(
                        traw_ps[:, 0:129],
                        lhsT=eus[e][:, t, :],
                        rhs=cxbs[e][:, ds(130 * t, 129)],
                        start=(t == 0),
                        stop=(t == NT - 1),
                    )
                c_sb = p_small.tile([128, 1], F32, tag="small", name=f"c_{e}")
                nc.vector.tensor_scalar_add(c_sb, traw_ps[:, 128:129], 1e-30)
                cinv = p_small.tile([128, 1], F32, tag="small", name=f"cinv_{e}")
                nc.vector.reciprocal(cinv, c_sb)
                nc.vector.tensor_scalar_mul(
                    rhss[e][:, 128:256], traw_ps[:, 0:128], cinv
                )

            def emit_abm(e):
                stg = p_stg.tile([128, NT, 257], BF16, tag="stg", name=f"stg_{e}")
                for pr in range(NT // 2):
                    ps = pp_abm.tile([128, 1024], F32, tag="pabm", name=f"abm_{e}_{pr}")
                    for k in range(2):
                        nc.tensor.matmul(
                            ps[:, ds(512 * k, 257)],
                            lhsT=eqs[e][:, ts(2 * pr + k, 128)],
                            rhs=rhss[e][:, 0:257],
                        )
                    src = bass.AP(
                        tensor=ps.tensor,
                        offset=ps.offset,
                        ap=[ps.ap[0], [512, 2], [1, 257]],
                    )
                    dst = stg[:, 2 * pr : 2 * pr + 2, :]
                    if pr % 2 == 0:
                        nc.vector.tensor_copy(dst, src)
                    else:
                        nc.scalar.copy(dst, src)
                stgs[e] = stg

            def emit_store(e):
                nc.sync.dma_start(
                    OUT[e].rearrange("p (t x) -> p t x", x=257), stgs[e]
                )

            # phase 1: all score matmuls + exps + transposes, PE never waits
            for e in range(B_LOC):
                emit_e1(e)
            # phase 2: traw runs one example ahead so the vector/scalar T'
            # latency is hidden behind the next traw on PE
            emit_traw(0)
            for e in range(B_LOC):
                if e + 1 < B_LOC:
                    emit_traw(e + 1)
                emit_abm(e)
                emit_store(e)

    nc.compile()
    return nc


_GRAPH = None


def _graph():
    global _GRAPH
    if _GRAPH is None:
        _GRAPH = _build_graph()
    return _GRAPH


def make_in_maps(C, Q, Cmask, Qmask, w_c, w_q, w_mul):
    """Shard full inputs into per-core input maps (host-side layout prep)."""
    C = np.asarray(C, dtype=np.float32)
    Q = np.asarray(Q, dtype=np.float32)
    wmul_r = np.asarray(w_mul, dtype=np.float32).reshape(D)
    wc_r = np.asarray(w_c, dtype=np.float32).reshape(D)
    wq_r = np.asarray(w_q, dtype=np.float32).reshape(D)
    in_maps = []
    for i in range(N_CORES):
        sl = slice(i * B_LOC, (i + 1) * B_LOC)
        Ci = C[sl]
        Qi = Q[sl]
        cmi = np.asarray(Cmask[sl], dtype=np.float32)  # [8, 1024]
        qneg = (np.asarray(Qmask[sl], dtype=np.float32) - 1.0) * 1e9  # [8, 128]
        # Qm'[e][d, j] = wmul[d]*Q[e,j,d] + wc[d], packed [128, 8*128] fp16
        qm = Qi.transpose(0, 2, 1) * wmul_r[None, :, None] + wc_r[None, :, None]
        qm = np.ascontiguousarray(
            qm.astype(np.float16).transpose(1, 0, 2).reshape(D, B_LOC * LQ)
        )
        # Q row-major, [j, e*128+d] bf16
        qs = np.ascontiguousarray(
            Qi.astype(ml_dtypes.bfloat16).transpose(1, 0, 2).reshape(LQ, B_LOC * D)
        )
        # p-major packed masked C: [e, p, t*130+x] = (cm*C)[128t+p, x] | cm | 0
        cmb = np.zeros((B_LOC, LC, 130), dtype=ml_dtypes.bfloat16)
        cmb[:, :, 0:128] = (Ci * cmi[:, :, None]).astype(ml_dtypes.bfloat16)
        cmb[:, :, 128] = cmi.astype(ml_dtypes.bfloat16)
        cmb = np.ascontiguousarray(
            cmb.reshape(B_LOC, NT, 128, 130)
            .transpose(0, 2, 1, 3)
            .reshape(B_LOC, 128, NT * 130)
        )
        # bias1[j, e] = (Q[e] @ wq)[j] + qneg[e, j]
        s1 = Qi @ wq_r  # [8, 128]
        b1 = np.ascontiguousarray((s1 + qneg).T.astype(np.float32))
        in_maps.append(
            {
                "CT": np.ascontiguousarray(Ci.transpose(0, 2, 1).astype(np.float16)),
                "QM": qm,
                "QS": qs,
                "CMB": cmb,
                "B1": b1,
            }
        )
    return in_maps


def assemble(results, C):
    """Gather per-core raw device outputs + input C into the full f32 output."""
    C = np.asarray(C, dtype=np.float32)
    out = np.empty((B, LC, 4 * D), dtype=np.float32)
    out[:, :, 0:D] = C
    for i in range(N_CORES):
        sl = slice(i * B_LOC, (i + 1) * B_LOC)
        o = np.asarray(results[i]["OUT"]).reshape(B_LOC, 128, NT, 257)
        o = o.astype(np.float32)
        a_raw = o[..., 0:128].transpose(0, 2, 1, 3).reshape(B_LOC, LC, D)
        b_raw = o[..., 128:256].transpose(0, 2, 1, 3).reshape(B_LOC, LC, D)
        r = o[..., 256].transpose(0, 2, 1).reshape(B_LOC, LC, 1)
        r = np.maximum(r, 1e-30)
        A = a_raw / r
        Bm = b_raw / r
        Ci = C[sl]
        out[sl, :, D : 2 * D] = A
        out[sl, :, 2 * D : 3 * D] = Ci * A
        out[sl, :, 3 * D : 4 * D] = Ci * Bm
    return out


def kernel(C, Q, Cmask, Qmask, w_c, w_q, w_mul, bias=None, **_ignored):
    # `bias` is mathematically a no-op: it shifts every score equally and
    # softmax is shift-invariant, so the output does not depend on it.
    nc = _graph()
    in_maps = make_in_maps(C, Q, Cmask, Qmask, w_c, w_q, w_mul)
    res = run_bass_kernel_spmd(nc, in_maps, core_ids=list(range(N_CORES)))
    return assemble(res.results, C)


# revision 14
# speedup vs baseline: 1.0125x; 1.0125x over previous
"""CQAttention Trainium2 kernel (8-core data parallel), v2.

Math (per example):
    S[i,j] = C@w_c [i] + Q@w_q [j] + (C*w_mul)@Q^T [i,j] + bias
    S1 = softmax_j(where(Qmask==0, -1e9, S))
    S2 = softmax_i(where(Cmask==0, -1e9, S))
    A  = S1 @ Q
    Bm = S1 @ S2^T @ C
    out = concat([C, A, C*A, C*Bm], axis=-1)

Key identities:
  - softmax shift-invariance: `bias` drops out; per-row offsets drop out of
    S1; per-column offsets drop out of S2.
  - With Qm'[d,j] = w_mul[d]*Q[j,d] + w_c[d] (host-packed, folds s0 into the
    score matmul) and bias1[j] = (Q@w_q)[j] + qneg[j] (host-packed):
        E^T[j,i] = exp(Qm'^T@C^T + bias1[j])    one matmul per example.
  - T' = S2^T@C normalized per row j is INVARIANT to any per-j scaling of
    the weights, so the S2 path can reuse E^T's values: the [Lc-part, Lq]
    layout needed for the Lc-contraction is produced by an XBAR DMA
    transpose of E^T (eu[p,t,j] = E^T[j, 128t+p]), not a second matmul.
    Masked-j columns give c[j]=0; +eps before the reciprocal keeps T'
    finite (=0) there, and those rows are killed by E^T[j,:]=0 in abm.
  - Row-major masked C for the T' contraction is built on-chip: XBAR
    transpose of C^T (fp16) -> crow, ones column memset, then one
    broadcast multiply by cm per example gives [cm*C | cm] in bf16.
  - abm per Lc-tile: [A_raw | Bm_raw | r] = E_tile^T.T @ [Q | T' | 1].
    Raw values + r are written out; the host divides by r and forms the
    C*A / C*Bm products during f32 assembly (elementwise O(Lc*D), same
    class as the host-side packing work).

Precision: scores fp16 (f32 PSUM), exp-weights bf16, outputs bf16 raw.
"""

import os
import sys
from contextlib import ExitStack

import ml_dtypes
import numpy as np

for _p in ("/opt/trn_rl_repo", "/root/.axon_site/_ro/trn_rl_repo"):
    if os.path.isdir(_p) and _p not in sys.path:
        sys.path.append(_p)

import concourse.bass as bass
import concourse.tile as tile
from concourse import bacc, mybir
from concourse.bass import ds, ts
from concourse.bass_utils import run_bass_kernel_spmd

F32 = mybir.dt.float32
FP16 = mybir.dt.float16
BF16 = mybir.dt.bfloat16
AF = mybir.ActivationFunctionType
ALU = mybir.AluOpType

N_CORES = 8
B, LC, LQ, D = 64, 1024, 128, 128
B_LOC = B // N_CORES  # 8 examples per core
NT = LC // 128  # 8 Lc tiles of 128


def _build_graph():
    nc = bacc.Bacc("TRN2", target_bir_lowering=False, debug=False)

    CT = nc.dram_tensor("CT", [B_LOC, D, LC], FP16, kind="ExternalInput").ap()
    QM = nc.dram_tensor("QM", [D, B_LOC * LQ], FP16, kind="ExternalInput").ap()
    QS = nc.dram_tensor("QS", [LQ, B_LOC * D], BF16, kind="ExternalInput").ap()
    # host-packed masked C, p-major: [e, p, t*130+x] = (cm*C)[128t+p, x] | cm | 0
    CMB = nc.dram_tensor("CMB", [B_LOC, 128, NT * 130], BF16, kind="ExternalInput").ap()
    B1 = nc.dram_tensor("B1", [LQ, B_LOC], F32, kind="ExternalInput").ap()
    # per-tile raw rows: OUT[e][m, t*257+n] = [A_raw | Bm_raw | r][128t+m, n]
    OUT = nc.dram_tensor("OUT", [B_LOC, 128, NT * 257], BF16, kind="ExternalOutput").ap()

    with tile.TileContext(nc) as tc:
        with ExitStack() as ctx:
            ep = ctx.enter_context

            const = ep(tc.tile_pool(name="const", bufs=1))
            p_ct = ep(tc.tile_pool(name="ct", bufs=B_LOC))
            p_cxb = ep(tc.tile_pool(name="cxb", bufs=4))  # pair tiles
            p_eq = ep(tc.tile_pool(name="eq", bufs=4))
            p_eu = ep(tc.tile_pool(name="eu", bufs=4))
            p_rhs = ep(tc.tile_pool(name="rhs", bufs=B_LOC))
            p_stg = ep(tc.tile_pool(name="stg", bufs=3))
            p_small = ep(tc.tile_pool(name="small", bufs=24))

            pp_e1 = ep(tc.tile_pool(name="pp_e1", bufs=2, space="PSUM"))
            pp_traw = ep(tc.tile_pool(name="pp_traw", bufs=2, space="PSUM"))
            pp_abm = ep(tc.tile_pool(name="pp_abm", bufs=2, space="PSUM"))

            qm_all = const.tile([D, B_LOC * LQ], FP16)
            nc.sync.dma_start(qm_all, QM)

            # PE warmup during the DMA head: dense matmuls flip HAM and start
            # the p-state ramp before real work arrives.
            warm_w = const.tile([128, 512], BF16)
            nc.vector.memset(warm_w, 1.0)
            for _ in range(10):
                warm_ps = pp_e1.tile([128, 512], F32, tag="pe1")
                nc.tensor.matmul(warm_ps[:, 0:256], lhsT=warm_w[:, 0:128], rhs=warm_w[:, 0:256])

            cts, cxbs, eqs, eus, rhss, stgs = {}, {}, {}, {}, {}, {}

            # ---- loads on the two hwdge queues only (swdge desc-gen on
            # gpsimd is slow and serializes with its compute ops) ----
            for e in range(B_LOC):
                ct = p_ct.tile([128, LC], FP16, tag="ct", name=f"ct_{e}")
                (nc.sync if e < 4 else nc.scalar).dma_start(ct, CT[e])
                cts[e] = ct
            qs_all = const.tile([LQ, B_LOC, D], BF16)
            nc.sync.dma_start(qs_all, QS.rearrange("p (e d) -> p e d", d=D))
            b1_sb = const.tile([LQ, B_LOC], F32)
            nc.sync.dma_start(b1_sb, B1)
            for pr in range(B_LOC // 2):
                cxb = p_cxb.tile([128, 2, NT * 130], BF16, tag="cxb", name=f"cxb_{pr}")
                nc.scalar.dma_start(
                    cxb, CMB.rearrange("(q e) p x -> q p e x", e=2)[pr]
                )
                cxbs[2 * pr] = cxb[:, 0, :]
                cxbs[2 * pr + 1] = cxb[:, 1, :]
            for e in range(B_LOC):
                # abm rhs = [Q | T' | 1]; Q block + ones col filled early
                rhs = p_rhs.tile([128, 260], BF16, tag="rhs", name=f"rhs_{e}")
                nc.gpsimd.tensor_copy(rhs[:, 0:128], qs_all[:, e, :])
                nc.gpsimd.memset(rhs[:, 256:257], 1.0)
                rhss[e] = rhs

            eq_batches = {}

            def emit_e1(e):
                # eq batched 4 examples per tile so one XBAR transpose covers
                # all four (transpose issue rate on sync would otherwise gate
                # the traw phase)
                if e % 2 == 0:
                    eq_batches[e // 2] = p_eq.tile(
                        [128, 2, LC], BF16, tag="eq", name=f"eqb_{e // 2}"
                    )
                eq = eq_batches[e // 2][:, e % 2, :]
                for h in range(2):
                    ps = pp_e1.tile([128, 512], F32, tag="pe1", name=f"e1ps_{e}_{h}")
                    nc.tensor.matmul(
                        ps, lhsT=qm_all[:, ts(e, LQ)], rhs=cts[e][:, ts(h, 512)]
                    )
                    nc.scalar.activation(
                        eq[:, ds(512 * h, 512)],
                        ps,
                        func=AF.Exp,
                        bias=b1_sb[:, e : e + 1],
                        scale=1.0,
                    )
                eqs[e] = eq
                if e % 2 == 1:
                    # eu[p, u, j] = E^T[j, 128u+p] over the 2-example batch
                    eu = p_eu.tile([128, 2 * NT, 128], BF16, tag="eu", name=f"eub_{e // 2}")
                    nc.sync.dma_start_transpose(
                        eu, eq_batches[e // 2].rearrange("p a x -> p (a x)")
                    )
                    for ee in (e - 1, e):
                        eus[ee] = eu[:, NT * (ee % 2) : NT * (ee % 2) + NT, :]

            def emit_traw(e):
                traw_ps = pp_traw.tile([128, 132], F32, tag="ptraw", name=f"traw_{e}")
                for t in range(NT):
                    nc.tensor.matmul(
                        traw_ps[:, 0:129],
                        lhsT=eus[e][:, t, :],
                        rhs=cxbs[e][:, ds(130 * t, 129)],
                        start=(t == 0),
                        stop=(t == NT - 1),
                    )
                c_sb = p_small.tile([128, 1], F32, tag="small", name=f"c_{e}")
                nc.vector.tensor_scalar_add(c_sb, traw_ps[:, 128:129], 1e-30)
                cinv = p_small.tile([128, 1], F32, tag="small", name=f"cinv_{e}")
                nc.vector.reciprocal(cinv, c_sb)
                nc.vector.tensor_scalar_mul(
                    rhss[e][:, 128:256], traw_ps[:, 0:128], cinv
                )

            def emit_abm(e):
                stg = p_stg.tile([128, NT, 257], BF16, tag="stg", name=f"stg_{e}")
                for pr in range(NT // 2):
                    ps = pp_abm.tile([128, 1024], F32, tag="pabm", name=f"abm_{e}_{pr}")
                    for k in range(2):
                        nc.tensor.matmul(
                            ps[:, ds(512 * k, 257)],
                            lhsT=eqs[e][:, ts(2 * pr + k, 128)],
                            rhs=rhss[e][:, 0:257],
                        )
                    src = bass.AP(
                        tensor=ps.tensor,
                        offset=ps.offset,
                        ap=[ps.ap[0], [512, 2], [1, 257]],
                    )
                    dst = stg[:, 2 * pr : 2 * pr + 2, :]
                    if pr % 2 == 0:
                        nc.vector.tensor_copy(dst, src)
                    else:
                        nc.scalar.copy(dst, src)
                stgs[e] = stg

            def emit_store(e):
                nc.sync.dma_start(
                    OUT[e].rearrange("p (t x) -> p t x", x=257), stgs[e]
                )

            # phase 1: all score matmuls + exps + transposes, PE never waits
            for e in range(B_LOC):
                emit_e1(e)
            # phase 2: traw runs one example ahead so the vector/scalar T'
            # latency is hidden behind the next traw on PE
            emit_traw(0)
            for e in range(B_LOC):
                if e + 1 < B_LOC:
                    emit_traw(e + 1)
                emit_abm(e)
                emit_store(e)

    nc.compile()
    return nc


_GRAPH = None


def _graph():
    global _GRAPH
    if _GRAPH is None:
        _GRAPH = _build_graph()
    return _GRAPH


def make_in_maps(C, Q, Cmask, Qmask, w_c, w_q, w_mul):
    """Shard full inputs into per-core input maps (host-side layout prep)."""
    C = np.asarray(C, dtype=np.float32)
    Q = np.asarray(Q, dtype=np.float32)
    wmul_r = np.asarray(w_mul, dtype=np.float32).reshape(D)
    wc_r = np.asarray(w_c, dtype=np.float32).reshape(D)
    wq_r = np.asarray(w_q, dtype=np.float32).reshape(D)
    in_maps = []
    for i in range(N_CORES):
        sl = slice(i * B_LOC, (i + 1) * B_LOC)
        Ci = C[sl]
        Qi = Q[sl]
        cmi = np.asarray(Cmask[sl], dtype=np.float32)  # [8, 1024]
        qneg = (np.asarray(Qmask[sl], dtype=np.float32) - 1.0) * 1e9  # [8, 128]
        # Qm'[e][d, j] = wmul[d]*Q[e,j,d] + wc[d], packed [128, 8*128] fp16
        qm = Qi.transpose(0, 2, 1) * wmul_r[None, :, None] + wc_r[None, :, None]
        qm = np.ascontiguousarray(
            qm.astype(np.float16).transpose(1, 0, 2).reshape(D, B_LOC * LQ)
        )
        # Q row-major, [j, e*128+d] bf16
        qs = np.ascontiguousarray(
            Qi.astype(ml_dtypes.bfloat16).transpose(1, 0, 2).reshape(LQ, B_LOC * D)
        )
        # p-major packed masked C: [e, p, t*130+x] = (cm*C)[128t+p, x] | cm | 0
        cmb = np.zeros((B_LOC, LC, 130), dtype=ml_dtypes.bfloat16)
        cmb[:, :, 0:128] = (Ci * cmi[:, :, None]).astype(ml_dtypes.bfloat16)
        cmb[:, :, 128] = cmi.astype(ml_dtypes.bfloat16)
        cmb = np.ascontiguousarray(
            cmb.reshape(B_LOC, NT, 128, 130)
            .transpose(0, 2, 1, 3)
            .reshape(B_LOC, 128, NT * 130)
        )
        # bias1[j, e] = (Q[e] @ wq)[j] + qneg[e, j]
        s1 = Qi @ wq_r  # [8, 128]
        b1 = np.ascontiguousarray((s1 + qneg).T.astype(np.float32))
        in_maps.append(
            {
                "CT": np.ascontiguousarray(Ci.transpose(0, 2, 1).astype(np.float16)),
                "QM": qm,
                "QS": qs,
                "CMB": cmb,
                "B1": b1,
            }
        )
    return in_maps


def assemble(results, C):
    """Gather per-core raw device outputs + input C into the full f32 output."""
    C = np.asarray(C, dtype=np.float32)
    out = np.empty((B, LC, 4 * D), dtype=np.float32)
    out[:, :, 0:D] = C
    for i in range(N_CORES):
        sl = slice(i * B_LOC, (i + 1) * B_LOC)
        o = np.asarray(results[i]["OUT"]).reshape(B_LOC, 128, NT, 257)
        o = o.astype(np.float32)
        a_raw = o[..., 0:128].transpose(0, 2, 1, 3).reshape(B_LOC, LC, D)
        b_raw = o[..., 128:256].transpose(0, 2, 1, 3).reshape(B_LOC, LC, D)
        r = o[..., 256].transpose(0, 2, 1).reshape(B_LOC, LC, 1)
        r = np.maximum(r, 1e-30)
        A = a_raw / r
        Bm = b_raw / r
        Ci = C[sl]
        out[sl, :, D : 2 * D] = A
        out[sl, :, 2 * D : 3 * D] = Ci * A
        out[sl, :, 3 * D : 4 * D] = Ci * Bm
    return out


def kernel(C, Q, Cmask, Qmask, w_c, w_q, w_mul, bias=None, **_ignored):
    # `bias` is mathematically a no-op: it shifts every score equally and
    # softmax is shift-invariant, so the output does not depend on it.
    nc = _graph()
    in_maps = make_in_maps(C, Q, Cmask, Qmask, w_c, w_q, w_mul)
    res = run_bass_kernel_spmd(nc, in_maps, core_ids=list(range(N_CORES)))
    return assemble(res.results, C)
